# revision 17
# baseline (speedup 1.0000x reference)
"""Trainium2 Bass kernel for ConfidenceCVXSelector.

Math: the reference builds A = fn fn^T (rank-2 Gram of row-normalized
(max_conf, dispersion) features), forms the normalized Laplacian
Ln = D~ - D^{-1/2} A D^{-1/2} and takes the Fiedler vector via dense eigh.

Because A is rank-2, Ln = I - G G^T with G = diag(dis) fn (dis = 1/sqrt(d),
d = fn @ s, s = sum_i fn_i). The non-trivial eigenvectors of Ln are G u for
eigenvectors u of the 2x2 matrix C = G^T G. s itself satisfies C s = s
(eigenvalue 1 <-> Ln eigenvalue 0), so the Fiedler vector is exactly
G u2 with u2 = perp(s) = (-S2, S1):

    fied_i = dis_i * (fn2_i * S1 - fn1_i * S2)

followed by the reference's sign canonicalization (flip so the largest-|.|
entry is positive) and min-max normalization - both invariant to the global
scale of fied, so no final renormalization is needed.

With mc = sigmoid(|x|) and v = (1-mc)/mc = exp(-|x|), the unnormalized
feature row is proportional to (1, u) with u = v*(1+v), so
fn1 = 1/sqrt(1+u^2), fn2 = u*fn1.

Per the sharding hint, the tiny reduced problem is solved redundantly:
the full 4096-element input is replicated to all 8 cores (each holds the
whole fn "shard" problem); core 0's output is returned. All compute is
O(N) elementwise + reductions on a single [128, 32] tile per core.
Cross-partition reductions/broadcasts run on the PE (ones-matmul for the
sums, identity-transpose + ones-matmul for max/min) - no GPSIMD ucode
library, and the scalar engine sees only the function order Exp -> Sqrt
so exactly two activation-table loads occur, both hidden behind the
input DMA / elementwise prologue.
"""

import sys

if "/opt/trn_rl_repo" not in sys.path:
    sys.path.insert(0, "/opt/trn_rl_repo")

import numpy as np

import concourse.bacc as bacc
import concourse.tile as tile
from concourse import mybir
from concourse.bass_utils import run_bass_kernel_spmd
from concourse.masks import make_identity

F32 = mybir.dt.float32
AF = mybir.ActivationFunctionType
ALU = mybir.AluOpType

P, FREE = 128, 32  # 4096 = 128 partitions x 32 free
N_CORES = 8

_CACHE = {}


def _build_nc():
    nc = bacc.Bacc("TRN2", target_bir_lowering=False)
    x_d = nc.dram_tensor("x", [P, FREE], F32, kind="ExternalInput")
    y_d = nc.dram_tensor("y", [P, FREE], F32, kind="ExternalOutput")

    with tile.TileContext(nc) as tc:
        with (
            tc.tile_pool(name="pool", bufs=1) as pool,
            tc.tile_pool(name="psum", bufs=1, space="PSUM") as psum,
        ):
            X = pool.tile([P, FREE], F32, tag="X")
            AB = pool.tile([P, FREE], F32, tag="AB")
            E = pool.tile([P, FREE], F32, tag="E")
            U = pool.tile([P, FREE], F32, tag="U")
            U2 = pool.tile([P, FREE], F32, tag="U2")
            U2P = pool.tile([P, FREE], F32, tag="U2P")
            QQ = pool.tile([P, FREE], F32, tag="QQ")
            FN1 = pool.tile([P, FREE], F32, tag="FN1")
            FN2 = pool.tile([P, FREE], F32, tag="FN2")
            WPRE = pool.tile([P, FREE], F32, tag="WPRE")
            DPRE = pool.tile([P, FREE], F32, tag="DPRE")
            D = pool.tile([P, FREE], F32, tag="D")
            DREC = pool.tile([P, FREE], F32, tag="DREC")
            DIS = pool.tile([P, FREE], F32, tag="DIS")
            W = pool.tile([P, FREE], F32, tag="W")
            FIED = pool.tile([P, FREE], F32, tag="FIED")
            OUT = pool.tile([P, FREE], F32, tag="OUT")

            R = pool.tile([P, 2], F32, tag="R")        # row sums (fn1, fn2)
            SB = pool.tile([P, 2], F32, tag="SB")      # bcast sums (S1, S2)
            PACK = pool.tile([P, 2], F32, tag="PACK")  # (rowmax, -rowmin)
            REDMM = pool.tile([2, 1], F32, tag="REDMM")  # (a, -b) on parts 0/1
            RHS3 = pool.tile([2, 3], F32, tag="RHS3")  # [[a,0,a],[0,-b,-b]]
            SBC3 = pool.tile([P, 3], F32, tag="SBC3")  # bcast (a, -b, a-b)

            SCL = pool.tile([P, 1], F32, tag="SCL")
            T1 = pool.tile([P, 1], F32, tag="T1")
            G1 = pool.tile([P, 1], F32, tag="G1")
            SIG = pool.tile([P, 1], F32, tag="SIG")
            SS = pool.tile([P, 1], F32, tag="SS")
            M1 = pool.tile([P, 1], F32, tag="M1")
            M2 = pool.tile([P, 1], F32, tag="M2")
            MS = pool.tile([P, 1], F32, tag="MS")

            # PE-side constants (built on GPSIMD, off the critical path)
            ONES = pool.tile([P, P], F32, tag="ONES")
            ID = pool.tile([P, P], F32, tag="ID")
            MASK = pool.tile([2, 3], F32, tag="MASK")  # [[1,0,1],[0,1,1]]

            SBP = psum.tile([P, 2], F32, tag="SBP")
            TP = psum.tile([2, P], F32, tag="TP")
            PBC3 = psum.tile([P, 3], F32, tag="PBC3")

            nc.gpsimd.memset(ONES[:, :], 1.0)
            make_identity(nc, ID[:, :])
            nc.gpsimd.memset(MASK[:, :], 1.0)
            nc.gpsimd.affine_select(
                out=MASK[:, :],
                in_=MASK[:, :],
                compare_op=ALU.not_equal,
                fill=0.0,
                base=-1,
                channel_multiplier=1,
                pattern=[[1, 3]],
            )

            # Load input
            nc.sync.dma_start(out=X[:, :], in_=x_d[:, :])

            # v = exp(-|x|); |x| by clearing the sign bit (exact).
            nc.vector.tensor_scalar(
                AB.bitcast(mybir.dt.uint32)[:, :],
                X.bitcast(mybir.dt.uint32)[:, :],
                0x7FFFFFFF,
                None,
                op0=ALU.bitwise_and,
            )
            nc.scalar.activation(E[:, :], AB[:, :], AF.Exp, scale=-1.0)

            # u = v*(1+v); qq = 1/(1+u^2); fn1 = sqrt(qq) (+ row sum via the
            # activation accumulator); fn2 = u*fn1 (+ row sum via DVE accum)
            nc.vector.scalar_tensor_tensor(
                U[:, :], in0=E[:, :], scalar=1.0, in1=E[:, :],
                op0=ALU.add, op1=ALU.mult,
            )
            nc.vector.tensor_tensor(U2[:, :], U[:, :], U[:, :], op=ALU.mult)
            nc.vector.tensor_scalar(U2P[:, :], U2[:, :], 1.0, None, op0=ALU.add)
            nc.vector.reciprocal(QQ[:, :], U2P[:, :])
            nc.scalar.activation(
                FN1[:, :], QQ[:, :], AF.Sqrt, accum_out=R[:, 0:1]
            )
            nc.vector.scalar_tensor_tensor(
                FN2[:, :], in0=U[:, :], scalar=1.0, in1=FN1[:, :],
                op0=ALU.mult, op1=ALU.mult, accum_out=R[:, 1:2],
            )

            # Global sums broadcast to all partitions in ONE matmul:
            # SBP = ones(128,128)^T @ R
            nc.tensor.matmul(SBP[:, :], ONES[:, :], R[:, :])
            nc.vector.tensor_copy(SB[:, :], SBP[:, :])

            # wpre = u*S1 - S2 ; dpre = u*S2 + S1 ; d = dpre*fn1
            nc.vector.tensor_scalar(
                WPRE[:, :], U[:, :], SB[:, 0:1], SB[:, 1:2],
                op0=ALU.mult, op1=ALU.subtract,
            )
            nc.vector.tensor_scalar(
                DPRE[:, :], U[:, :], SB[:, 1:2], SB[:, 0:1],
                op0=ALU.mult, op1=ALU.add,
            )
            nc.vector.tensor_tensor(D[:, :], DPRE[:, :], FN1[:, :], op=ALU.mult)
            # w = wpre*fn1 overlaps; dis = sqrt(1/d); fied = w*dis
            nc.vector.tensor_tensor(W[:, :], WPRE[:, :], FN1[:, :], op=ALU.mult)
            nc.vector.reciprocal(DREC[:, :], D[:, :])
            nc.scalar.activation(DIS[:, :], DREC[:, :], AF.Sqrt)
            nc.vector.tensor_tensor(FIED[:, :], W[:, :], DIS[:, :], op=ALU.mult)

            # Row max and negated row min
            nc.vector.tensor_reduce(
                PACK[:, 0:1], FIED[:, :], axis=mybir.AxisListType.X, op=ALU.max
            )
            nc.vector.tensor_reduce(
                PACK[:, 1:2], FIED[:, :], axis=mybir.AxisListType.X, op=ALU.min,
                negate=True,
            )

            # Global a = max, nb = -min: transpose -> free-dim max reduce.
            nc.tensor.transpose(TP[:, :], PACK[:, :], ID[:, :])
            nc.vector.tensor_reduce(
                REDMM[:, :], TP[:, :], axis=mybir.AxisListType.X, op=ALU.max
            )
            # Broadcast (a, nb, a+nb) to all partitions in one matmul:
            # rhs = MASK * [a;nb] -> [[a,0,a],[0,nb,nb]]; ones(2,128)^T @ rhs.
            nc.vector.tensor_tensor(
                RHS3[:, :], MASK[:, :], REDMM[:, 0:1].broadcast_to([2, 3]),
                op=ALU.mult,
            )
            nc.tensor.matmul(PBC3[:, :], ONES[0:2, :], RHS3[:, :])
            nc.vector.tensor_copy(SBC3[:, :], PBC3[:, :])

            # sigma = +1 if a+b >= 0 else -1 (argmax-|.| sign canonicalization)
            # min(sigma*a, sigma*b) = g*(a+b) - a  with g = (a+b >= 0).
            # All on [128,1] broadcast lanes; out = fied*(sigma*scl) - m'*scl.
            nc.vector.reciprocal(SCL[:, :], SBC3[:, 2:3])
            nc.vector.tensor_tensor(
                T1[:, :], SBC3[:, 0:1], SBC3[:, 1:2], op=ALU.subtract
            )
            nc.vector.tensor_scalar(G1[:, :], T1[:, :], 0.0, None, op0=ALU.is_ge)
            nc.vector.tensor_scalar(
                SIG[:, :], G1[:, :], 2.0, 1.0, op0=ALU.mult, op1=ALU.subtract
            )
            nc.vector.tensor_tensor(SS[:, :], SIG[:, :], SCL[:, :], op=ALU.mult)
            nc.vector.tensor_tensor(M1[:, :], G1[:, :], T1[:, :], op=ALU.mult)
            nc.vector.tensor_tensor(
                M2[:, :], M1[:, :], SBC3[:, 0:1], op=ALU.subtract
            )
            nc.vector.tensor_tensor(MS[:, :], M2[:, :], SCL[:, :], op=ALU.mult)

            nc.vector.tensor_scalar(
                OUT[:, :], FIED[:, :], SS[:, 0:1], MS[:, 0:1],
                op0=ALU.mult, op1=ALU.subtract,
            )

            nc.sync.dma_start(out=y_d[:, :], in_=OUT[:, :])

    nc.compile()
    return nc


def kernel(**inputs: np.ndarray) -> np.ndarray:
    x = np.ascontiguousarray(np.asarray(inputs["pred_logits"], dtype=np.float32))
    b, c, h, w = x.shape  # (1, 1, 64, 64)
    x2d = x.reshape(P, FREE)

    if "nc" not in _CACHE:
        _CACHE["nc"] = _build_nc()
    nc = _CACHE["nc"]

    in_maps = [{"x": x2d} for _ in range(N_CORES)]
    res = run_bass_kernel_spmd(nc, in_maps, core_ids=list(range(N_CORES)))
    out = np.asarray(res.results[0]["y"], dtype=np.float32)
    return out.reshape(b, c, h, w)


if __name__ == "__main__":
    rng = np.random.default_rng(0)
    x = rng.standard_normal((1, 1, 64, 64), dtype=np.float32)
    y = kernel(pred_logits=x)
    print("kernel out", y.shape, y.dtype, y.min(), y.max())
